# revision 1
# baseline (speedup 1.0000x reference)
"""DendriteLayer Trainium2 kernel.

Math (reference): out0 = x @ (w_in*w_in_mask).T + b_in; a = out0.reshape(B, dpc, out_dim);
winner = argmax_d(a * boost); out1 = a * one_hot(winner); y = out1f @ (w_out*dend_mask).T + b_out.

Sharding: 8 cores, core c owns global units u in [c*256, (c+1)*256) (all dpc=8 dendrites)
and output columns v with (v % 256) in [c*32, (c+1)*32). Both k-winners and the
block-diagonal output stage are then fully local to a core (no collectives).

Per-core j' layout is u'-major interleaved: j' = u'*8 + d, so the 8 dendrites of a
unit are consecutive, and each 512-wide chunk of j' is self-contained for both the
k-winners (max over d) and the output segment-sums.

The big matmul runs as a 3-term float32r hi/lo split (Xr@Wr + Xr@Wl + Xl@Wr), which
is fp32-accurate (verified ~1.7e-7 rel) at 1 PE cycle/row instead of fp32's 4.
Both splits happen on device; weight prep (mask multiply + hi/lo split) is
per-kt strips so the next chunk's prep overlaps the current chunk's tail
matmuls. The k-winners boost is applied post-matmul in stage-2 (gb = G*boost
for the argmax, z = G*W_elem for the values), keeping weight prep free of any
exp/table dependency.
"""

import numpy as np

B, IN_DIM, OUT_DIM, DPC = 4096, 2048, 2048, 8
ND = OUT_DIM * DPC
NCORES = 8
UPC = OUT_DIM // NCORES          # units per core = 256
JPC = UPC * DPC                  # j' per core = 2048
CHUNK = 512                      # j' chunk width (64 units x 8 dendrites)
NCHUNK = JPC // CHUNK            # 4
BT = 128                         # batch tile
NBT = B // BT                    # 32
KT = 128                         # k tile
NKT = IN_DIM // KT               # 16
YW = CHUNK // DPC                # y columns per chunk = 64
BOOST_STRENGTH = 2.0

_prog_cache = {}
LAST_RESULTS = None


def _round_f32r(a):
    """Round fp32 -> f32r (11 explicit mantissa bits), RNE. Exact bit-twiddle."""
    u = a.view(np.uint32).astype(np.uint64)
    u = u + np.uint64(0xFFF) + ((u >> np.uint64(12)) & np.uint64(1))
    u = u & np.uint64(0xFFFFF000)
    return u.astype(np.uint32).view(np.float32)


def _build(has_bin, has_bout):
    import concourse.mybir as mybir
    import concourse.tile as tile
    from concourse import bacc

    f32 = mybir.dt.float32
    f32r = mybir.dt.float32r

    nc = bacc.Bacc("TRN2", target_bir_lowering=False, debug=False)
    XT_d = nc.dram_tensor("XT", [IN_DIM, B], f32, kind="ExternalInput").ap()
    WT_d = nc.dram_tensor("WT", [IN_DIM, JPC], f32, kind="ExternalInput").ap()
    MT_d = nc.dram_tensor("MT", [IN_DIM, JPC], f32, kind="ExternalInput").ap()
    We_d = nc.dram_tensor("We", [128, JPC], f32, kind="ExternalInput").ap()
    duty_d = nc.dram_tensor("duty", [128, JPC], f32, kind="ExternalInput").ap()
    if has_bin:
        bin_d = nc.dram_tensor("bin", [128, JPC], f32, kind="ExternalInput").ap()
    if has_bout:
        bout_d = nc.dram_tensor("bout", [128, NCHUNK * YW], f32, kind="ExternalInput").ap()
    Y_d = nc.dram_tensor("Y", [NCHUNK, B, YW], f32, kind="ExternalOutput").ap()

    with tile.TileContext(nc) as tc:
        with tc.tile_pool(name="tables", bufs=1) as tbl, \
             tc.tile_pool(name="wres2", bufs=2) as wres2, \
             tc.tile_pool(name="wres1", bufs=1) as wres1, \
             tc.tile_pool(name="wstrip", bufs=3) as wstrip, \
             tc.tile_pool(name="xio", bufs=2) as xio, \
             tc.tile_pool(name="xsplit", bufs=2) as xsplit, \
             tc.tile_pool(name="ypool", bufs=3) as ypool, \
             tc.tile_pool(name="st2", bufs=2) as st2, \
             tc.tile_pool(name="psum", bufs=8, space="PSUM") as psum:

            # ---- one-time tables ----
            du = tbl.tile([128, JPC], f32, name="du", tag="du")
            nc.sync.dma_start(du[:], duty_d[:])
            bo = tbl.tile([128, JPC], f32, name="bo")  # boost, broadcast on partitions
            bias_t = tbl.tile([128, 1], f32, name="bias_t")
            nc.gpsimd.memset(bias_t[:], BOOST_STRENGTH / DPC)
            scale_t = tbl.tile([128, 1], f32, name="scale_t")
            nc.gpsimd.memset(scale_t[:], -BOOST_STRENGTH)
            nc.scalar.activation(bo[:], du[:], mybir.ActivationFunctionType.Exp,
                                 bias=bias_t[:], scale=scale_t[:])
            if has_bin:
                bbt = tbl.tile([128, JPC], f32, name="bbt")
                nc.sync.dma_start(bbt[:], bin_d[:])  # plain b_in (boost applied in stage-2)
            if has_bout:
                bot = tbl.tile([128, NCHUNK * YW], f32, name="bot")
                nc.sync.dma_start(bot[:], bout_d[:])

            NDB = 8  # kt strips 0..NDB-1 double-buffered, prepped in prev chunk's tail

            def emit_strip_prep(w, kt, mode="mixed"):
                ws = wstrip.tile([128, CHUNK], f32, name=f"ws_{w}_{kt}", tag="ws")
                ms = wstrip.tile([128, CHUNK], f32, name=f"ms_{w}_{kt}", tag="ms")
                nc.scalar.dma_start(ws[:], WT_d[kt*KT:(kt+1)*KT, w*CHUNK:(w+1)*CHUNK])
                nc.sync.dma_start(ms[:], MT_d[kt*KT:(kt+1)*KT, w*CHUNK:(w+1)*CHUNK])
                meng = nc.vector if mode == "dve" else nc.gpsimd
                seng = nc.gpsimd if mode == "gps" else nc.vector
                meng.tensor_mul(ws[:], ws[:], ms[:])
                pool = wres2 if kt < NDB else wres1
                wr = pool.tile([128, CHUNK], f32r, name=f"wr_{w}_{kt}", tag=f"wr{kt}")
                wl = pool.tile([128, CHUNK], f32r, name=f"wl_{w}_{kt}", tag=f"wl{kt}")
                seng.tensor_copy(wr[:], ws[:])
                seng.tensor_sub(wl[:], ws[:], wr[:].bitcast(f32))
                return wr, wl

            def emit_x(w, i, split_dma=False):
                xf = xio.tile([128, NKT * BT], f32, name=f"xf_{w}_{i}", tag="xf")
                src_ap = XT_d[:, i*BT:(i+1)*BT].rearrange("(kt p) b -> p kt b", p=128)
                dst_ap = xf[:].rearrange("p (kt b) -> p kt b", b=BT)
                h = NKT // 2
                nc.sync.dma_start(dst_ap[:, :h, :], src_ap[:, :h, :])
                nc.scalar.dma_start(dst_ap[:, h:, :], src_ap[:, h:, :])
                xr = xsplit.tile([128, NKT * BT], f32r, name=f"xr_{w}_{i}", tag="xr")
                xl = xsplit.tile([128, NKT * BT], f32r, name=f"xl_{w}_{i}", tag="xl")
                nc.vector.tensor_copy(xr[:], xf[:])
                nc.vector.tensor_sub(xl[:], xf[:], xr[:].bitcast(f32))
                return xr, xl

            strips = {}  # (w, kt) -> (wr, wl)
            for w in range(NCHUNK):
                if w == 0:
                    xpre = emit_x(0, 0, split_dma=True)
                    xpre1 = emit_x(0, 1, split_dma=True)
                    for kt in range(NKT):
                        strips[(0, kt)] = emit_strip_prep(0, kt, "dve" if kt % 2 == 0 else "mixed")
                    # raw W_elem table; boost is applied in stage-2 instead of
                    # being folded into the weights (keeps strip prep off the
                    # exp/boost critical path at startup and boundaries)
                    we = tbl.tile([128, JPC], f32, name="we")
                    nc.sync.dma_start(we[:], We_d[:])
                else:
                    for kt in range(NDB, NKT):
                        strips[(w, kt)] = emit_strip_prep(w, kt)

                Vw = we[:, w*CHUNK:(w+1)*CHUNK]

                xnext = None
                for i in range(NBT):
                    if w == 0 and i == 0:
                        xr, xl = xpre
                    elif w == 0 and i == 1:
                        xr, xl = xpre1
                    elif xnext is not None:
                        xr, xl = xnext
                    else:
                        xr, xl = emit_x(w, i)

                    # ---- matmul: G = sum_k XrWr + XrWl + XlWr ----
                    g = psum.tile([128, CHUNK], f32, name=f"g_{w}_{i}", tag="g")
                    nmm = 3 * NKT
                    n = 0
                    for kt in range(NKT):
                        lr = xr[:, kt*BT:(kt+1)*BT]
                        ll = xl[:, kt*BT:(kt+1)*BT]
                        wr, wl = strips[(w, kt)]
                        nc.tensor.matmul(g[:], lr, wr[:], start=(n == 0), stop=(n == nmm-1)); n += 1
                        nc.tensor.matmul(g[:], lr, wl[:], start=False, stop=(n == nmm-1)); n += 1
                        nc.tensor.matmul(g[:], ll, wr[:], start=False, stop=(n == nmm-1)); n += 1

                    # prefetch next b-tile's X ahead of stage-2 queue traffic
                    nxt = i + 1
                    if nxt < NBT and not (w == 0 and nxt <= 1):
                        xnext = emit_x(w, nxt)
                    else:
                        xnext = None

                    # ---- stage 2: k-winners + masked output segment-sum ----
                    if has_bin:
                        gs = st2.tile([128, CHUNK], f32, name=f"gs_{w}_{i}", tag="gs")
                        nc.vector.tensor_add(gs[:], g[:], bbt[:, w*CHUNK:(w+1)*CHUNK])
                        gin = gs
                    else:
                        gin = g
                    gb = st2.tile([128, CHUNK], f32, name=f"gb_{w}_{i}", tag="gb")
                    nc.vector.tensor_mul(gb[:], gin[:], bo[:, w*CHUNK:(w+1)*CHUNK])
                    m = st2.tile([128, CHUNK // DPC], f32, name=f"m_{w}_{i}", tag="m")
                    nc.vector.reduce_max(m[:], gb[:].rearrange("p (u d) -> p u d", d=DPC),
                                         axis=mybir.AxisListType.X)
                    e = st2.tile([128, CHUNK], f32, name=f"e_{w}_{i}", tag="e")
                    mb = m[:].rearrange("p (u one) -> p u one", one=1).broadcast_to((128, CHUNK // DPC, DPC))
                    nc.vector.tensor_tensor(e[:].rearrange("p (u d) -> p u d", d=DPC),
                                            gb[:].rearrange("p (u d) -> p u d", d=DPC),
                                            mb, op=mybir.AluOpType.is_ge)
                    z = st2.tile([128, CHUNK], f32, name=f"z_{w}_{i}", tag="z")
                    nc.vector.tensor_mul(z[:], gin[:], Vw)
                    nc.gpsimd.tensor_mul(z[:], z[:], e[:])
                    # y64[p, 8s+q] = sum_t z[64s + 8t + q]
                    y = ypool.tile([128, YW], f32, name=f"y_{w}_{i}", tag="y")
                    ov = z[:].rearrange("p (s t q) -> p s q t", s=8, t=8, q=8)
                    yv = y[:].rearrange("p (s q) -> p s q", q=8)
                    nc.vector.reduce_sum(yv, ov, axis=mybir.AxisListType.X)
                    if has_bout:
                        nc.vector.tensor_add(y[:], y[:], bot[:, w*YW:(w+1)*YW])
                    nc.scalar.dma_start(Y_d[w, i*BT:(i+1)*BT, :], y[:])

                    # pre-emit next chunk's double-buffered strip preps in our tail
                    if w + 1 < NCHUNK and NBT - NDB <= i + 1:
                        kt = i + 1 - (NBT - NDB)
                        if kt < NDB:
                            strips[(w + 1, kt)] = emit_strip_prep(w + 1, kt)

    nc.compile()
    return nc


def kernel(x, w_in, b_in, w_in_mask, w_out, b_out, duty_cycle):
    from concourse.bass_utils import run_bass_kernel_spmd
    global LAST_RESULTS

    x = np.ascontiguousarray(x, dtype=np.float32)
    w_in = np.asarray(w_in, dtype=np.float32)
    w_in_mask = np.asarray(w_in_mask, dtype=np.float32)
    w_out = np.asarray(w_out, dtype=np.float32)
    b_in = np.asarray(b_in, dtype=np.float32)
    b_out = np.asarray(b_out, dtype=np.float32)
    duty_cycle = np.asarray(duty_cycle, dtype=np.float32)
    assert x.shape == (B, IN_DIM) and w_in.shape == (ND, IN_DIM)

    has_bin = bool(np.any(b_in))
    has_bout = bool(np.any(b_out))

    key = (has_bin, has_bout)
    if key not in _prog_cache:
        _prog_cache[key] = _build(has_bin, has_bout)
    nc = _prog_cache[key]

    XT = np.ascontiguousarray(x.T)                       # [IN_DIM, B]
    # w_in[d*OUT + c*UPC + u', k] -> per-core [k, j'=u'*8+d] via reshape/transpose
    w4 = w_in.reshape(DPC, NCORES, UPC, IN_DIM)          # [d, c, u', k]
    m4 = w_in_mask.reshape(DPC, NCORES, UPC, IN_DIM)
    wof = w_out.reshape(-1)

    uprime = np.arange(UPC)
    dd = np.arange(DPC)
    jp_u = np.repeat(uprime, DPC)                        # u'(j') ; j' = u'*8 + d
    jp_d = np.tile(dd, UPC)                              # d(j')

    in_maps = []
    for c in range(NCORES):
        rows = jp_d * OUT_DIM + c * UPC + jp_u           # global w_in row per j'
        WT = np.ascontiguousarray(w4[:, c].transpose(2, 1, 0).reshape(IN_DIM, JPC))
        MT = np.ascontiguousarray(m4[:, c].transpose(2, 1, 0).reshape(IN_DIM, JPC))
        v = jp_d * (OUT_DIM // DPC) + c * (UPC // DPC) + (jp_u // DPC)  # d*256 + c*32 + u'//8
        t = jp_u % DPC
        We = np.broadcast_to(wof[v * ND + v * DPC + t].astype(np.float32), (128, JPC))
        duty = np.broadcast_to(duty_cycle[jp_d, c * UPC + jp_u].astype(np.float32), (128, JPC))
        im = {"XT": XT, "WT": WT, "MT": MT, "We": np.ascontiguousarray(We),
              "duty": np.ascontiguousarray(duty)}
        if has_bin:
            im["bin"] = np.ascontiguousarray(np.broadcast_to(b_in[rows], (128, JPC)))
        if has_bout:
            # bout4[w*64 + s*8 + q] = b_out[v], v = q*256 + c*32 + 8w + s
            wq = np.arange(NCHUNK * YW)
            wi, si, qi = wq // YW, (wq % YW) // 8, wq % 8
            vv = qi * (OUT_DIM // DPC) + c * (UPC // DPC) + 8 * wi + si
            im["bout"] = np.ascontiguousarray(np.broadcast_to(b_out[vv], (128, NCHUNK * YW)))
        in_maps.append(im)

    import os
    trace = bool(os.environ.get("KERNEL_TRACE"))
    last_err = None
    for _attempt in range(3):
        try:
            res = run_bass_kernel_spmd(nc, in_maps, list(range(NCORES)), trace=trace)
            break
        except Exception as err:  # rare transient device fault on first execute
            last_err = err
            import time as _time
            _time.sleep(2.0)
    else:
        raise last_err
    LAST_RESULTS = res

    # Y4[w, b, s*8+q] (per core) -> y[b, q*256 + c*32 + 8w + s]
    Yc = np.stack([res.results[c]["Y"] for c in range(NCORES)], axis=0)  # [8, NCHUNK, B, 64]
    Yc = Yc.reshape(NCORES, NCHUNK, B, 8, 8)             # [c, w, b, s, q]
    y = Yc.transpose(2, 4, 0, 1, 3).reshape(B, OUT_DIM)  # [b, q, c, w, s] -> v = q*256+c*32+8w+s
    return np.ascontiguousarray(y)



# revision 7
# speedup vs baseline: 2.6387x; 2.6387x over previous
"""DendriteLayer Trainium2 kernel.

Math (reference): out0 = x @ (w_in*w_in_mask).T + b_in; a = out0.reshape(B, dpc, out_dim);
winner = argmax_d(a * boost); out1 = a * one_hot(winner); y = out1f @ (w_out*dend_mask).T + b_out.

Sharding: 8 cores, core c owns global units u in [c*256, (c+1)*256) (all dpc=8 dendrites)
and output columns v with (v % 256) in [c*32, (c+1)*32). Both k-winners and the
block-diagonal output stage are then fully local to a core (no collectives).

Per-core j' layout is u'-major interleaved: j' = u'*8 + d, so the 8 dendrites of a
unit are consecutive, and each 512-wide chunk of j' is self-contained for both the
k-winners (max over d) and the output segment-sums.

The matmul runs as a SINGLE f32r (11-mantissa-bit) term: host pre-folds the sparsity
mask AND the k-winners boost into the weights (WB = rne11(w_in*mask*boost)), and
pre-rounds X to f32r. The PE then computes G_b = Xr @ WB directly boosted, so the
argmax needs no separate boost multiply, and the winner values come from
z = G_b * (w_out_elem / boost) with the boost divided out host-side. This cuts PE
work 3x vs an fp32-accurate hi/lo split; the f32r rounding perturbs the argmax for
~1.2e-4 of units, giving rel_err ~1.1e-2 (CPU-simulated, gate is 2e-2).

All weights stay resident in SBUF (16 f32r kt-strips, 128KB/partition), so X is
streamed exactly once (32MB) and total HBM traffic is ~53MB/core vs ~160MB for a
chunk-looped X. Stage-2 (max/is_ge/mul/segment-sum) reads G straight from PSUM.
"""

import numpy as np

B, IN_DIM, OUT_DIM, DPC = 4096, 2048, 2048, 8
ND = OUT_DIM * DPC
NCORES = 8
UPC = OUT_DIM // NCORES          # units per core = 256
JPC = UPC * DPC                  # j' per core = 2048
CHUNK = 512                      # j' chunk width (64 units x 8 dendrites)
NCHUNK = JPC // CHUNK            # 4
BT = 128                         # batch tile
NBT = B // BT                    # 32
KT = 128                         # k tile
NKT = IN_DIM // KT               # 16
YW = CHUNK // DPC                # y columns per chunk = 64
BOOST_STRENGTH = 2.0

_prog_cache = {}
LAST_RESULTS = None


def _round_f32r(a):
    """Round fp32 -> f32r (11 explicit mantissa bits), RNE. Values stay exactly
    representable so the PE's own f32r read rounding is a no-op."""
    u = np.ascontiguousarray(a, dtype=np.float32).view(np.uint32).astype(np.uint64)
    u = u + np.uint64(0x7FF) + ((u >> np.uint64(12)) & np.uint64(1))
    u = u & np.uint64(0xFFFFF000)
    return u.astype(np.uint32).view(np.float32)


def _build(has_bin, has_bout):
    import concourse.mybir as mybir
    import concourse.tile as tile
    from concourse import bacc

    f32 = mybir.dt.float32
    f32r = mybir.dt.float32r

    nc = bacc.Bacc("TRN2", target_bir_lowering=False, debug=False)
    XT_d = nc.dram_tensor("XT", [IN_DIM, B], f32r, kind="ExternalInput").ap()
    WT_d = nc.dram_tensor("WT", [IN_DIM, JPC], f32r, kind="ExternalInput").ap()
    We_d = nc.dram_tensor("We", [128, JPC], f32, kind="ExternalInput").ap()
    if has_bin:
        binb_d = nc.dram_tensor("binb", [128, JPC], f32, kind="ExternalInput").ap()
    if has_bout:
        bout_d = nc.dram_tensor("bout", [128, NCHUNK * YW], f32, kind="ExternalInput").ap()
    Y_d = nc.dram_tensor("Y", [B, NCHUNK, YW], f32, kind="ExternalOutput").ap()

    with tile.TileContext(nc) as tc:
        with tc.tile_pool(name="wres", bufs=1) as wres, \
             tc.tile_pool(name="tbl", bufs=1) as tbl, \
             tc.tile_pool(name="xio", bufs=3) as xio, \
             tc.tile_pool(name="st2", bufs=2) as st2, \
             tc.tile_pool(name="ypool", bufs=3) as ypool, \
             tc.tile_pool(name="psum", bufs=8, space="PSUM") as psum:

            # ---- one-time tables (idle gpsimd queue) ----
            we = tbl.tile([128, JPC], f32, name="we")
            nc.gpsimd.dma_start(we[:], We_d[:])
            if has_bin:
                binb = tbl.tile([128, JPC], f32, name="binb")
                nc.gpsimd.dma_start(binb[:], binb_d[:])
            if has_bout:
                bout = tbl.tile([128, NCHUNK * YW], f32, name="bout")
                nc.gpsimd.dma_start(bout[:], bout_d[:])

            # ---- resident masked+boosted f32r weights, 16 kt strips ----
            wt = []
            dma_engs = [nc.sync, nc.scalar, nc.gpsimd]
            for kt in range(NKT):
                w_ = wres.tile([128, JPC], f32r, name=f"w{kt}", tag=f"w{kt}")
                dma_engs[kt % 3].dma_start(w_[:], WT_d[kt * KT:(kt + 1) * KT, :])
                wt.append(w_)

            for i in range(NBT):
                xf = xio.tile([128, NKT * BT], f32r, name=f"xf_{i}", tag="xf")
                src = XT_d[:, i * BT:(i + 1) * BT].rearrange("(kt p) b -> p kt b", p=128)
                dst = xf[:].rearrange("p (kt b) -> p kt b", b=BT)
                h = NKT // 2
                nc.sync.dma_start(dst[:, :h, :], src[:, :h, :])
                nc.scalar.dma_start(dst[:, h:, :], src[:, h:, :])

                # ---- matmul: G_b[b, j'] accumulated over 16 kt strips ----
                g = [psum.tile([128, CHUNK], f32, name=f"g_{i}_{w}", tag="g")
                     for w in range(NCHUNK)]
                for kt in range(NKT):
                    lhsT = xf[:, kt * BT:(kt + 1) * BT]
                    for w in range(NCHUNK):
                        nc.tensor.matmul(g[w][:], lhsT, wt[kt][:, w * CHUNK:(w + 1) * CHUNK],
                                         start=(kt == 0), stop=(kt == NKT - 1))

                # ---- stage 2: k-winners + masked output segment-sum ----
                y = ypool.tile([128, NCHUNK * YW], f32, name=f"y_{i}", tag="y")
                for w in range(NCHUNK):
                    if has_bin:
                        gs = st2.tile([128, CHUNK], f32, name=f"gs_{i}_{w}", tag="gs")
                        nc.vector.tensor_add(gs[:], g[w][:], binb[:, w * CHUNK:(w + 1) * CHUNK])
                        gin = gs
                    else:
                        gin = g[w]
                    m = st2.tile([128, CHUNK // DPC], f32, name=f"m_{i}_{w}", tag="m")
                    nc.vector.reduce_max(m[:], gin[:].rearrange("p (u d) -> p u d", d=DPC),
                                         axis=mybir.AxisListType.X)
                    e = st2.tile([128, CHUNK], f32, name=f"e_{i}_{w}", tag="e")
                    mb = m[:].rearrange("p (u one) -> p u one", one=1).broadcast_to(
                        (128, CHUNK // DPC, DPC))
                    nc.vector.tensor_tensor(e[:].rearrange("p (u d) -> p u d", d=DPC),
                                            gin[:].rearrange("p (u d) -> p u d", d=DPC),
                                            mb, op=mybir.AluOpType.is_ge)
                    z = st2.tile([128, CHUNK], f32, name=f"z_{i}_{w}", tag="z")
                    nc.vector.tensor_mul(z[:], gin[:], we[:, w * CHUNK:(w + 1) * CHUNK])
                    nc.gpsimd.tensor_mul(z[:], z[:], e[:])
                    # y64[p, 8s+q] = sum_t z[64s + 8t + q]
                    ov = z[:].rearrange("p (s t q) -> p s q t", s=8, t=8, q=8)
                    yv = y[:, w * YW:(w + 1) * YW].rearrange("p (s q) -> p s q", q=8)
                    nc.vector.reduce_sum(yv, ov, axis=mybir.AxisListType.X)
                if has_bout:
                    nc.vector.tensor_add(y[:], y[:], bout[:])
                nc.gpsimd.dma_start(
                    Y_d[i * BT:(i + 1) * BT, :, :].rearrange("b w yy -> b (w yy)"), y[:])

    nc.compile()
    return nc


def kernel(x, w_in, b_in, w_in_mask, w_out, b_out, duty_cycle):
    from concourse.bass_utils import run_bass_kernel_spmd
    global LAST_RESULTS

    x = np.ascontiguousarray(x, dtype=np.float32)
    w_in = np.asarray(w_in, dtype=np.float32)
    w_in_mask = np.asarray(w_in_mask, dtype=np.float32)
    w_out = np.asarray(w_out, dtype=np.float32)
    b_in = np.asarray(b_in, dtype=np.float32)
    b_out = np.asarray(b_out, dtype=np.float32)
    duty_cycle = np.asarray(duty_cycle, dtype=np.float32)
    assert x.shape == (B, IN_DIM) and w_in.shape == (ND, IN_DIM)

    has_bin = bool(np.any(b_in))
    has_bout = bool(np.any(b_out))

    key = (has_bin, has_bout)
    if key not in _prog_cache:
        _prog_cache[key] = _build(has_bin, has_bout)
    nc = _prog_cache[key]

    boost = np.exp((1.0 / DPC - duty_cycle) * BOOST_STRENGTH).astype(np.float32)  # [DPC, OUT_DIM]
    XT = np.ascontiguousarray(_round_f32r(x).T)          # [IN_DIM, B], f32r values

    # w_in[d*OUT + c*UPC + u', k] -> per-core [k, j'=u'*8+d] via reshape/transpose
    w4 = w_in.reshape(DPC, NCORES, UPC, IN_DIM)          # [d, c, u', k]
    m4 = w_in_mask.reshape(DPC, NCORES, UPC, IN_DIM)
    wof = w_out.reshape(-1)

    uprime = np.arange(UPC)
    dd = np.arange(DPC)
    jp_u = np.repeat(uprime, DPC)                        # u'(j') ; j' = u'*8 + d
    jp_d = np.tile(dd, UPC)                              # d(j')

    in_maps = []
    for c in range(NCORES):
        bc = boost[:, c * UPC:(c + 1) * UPC]             # [d, u']
        WTc = (w4[:, c] * m4[:, c]) * bc[:, :, None]     # masked + boosted, [d, u', k]
        WT = _round_f32r(WTc.transpose(2, 1, 0).reshape(IN_DIM, JPC))
        v = jp_d * (OUT_DIM // DPC) + c * (UPC // DPC) + (jp_u // DPC)  # d*256 + c*32 + u'//8
        t = jp_u % DPC
        bcol = boost[jp_d, c * UPC + jp_u]               # boost per j' column
        We = wof[v * ND + v * DPC + t].astype(np.float32) / bcol
        im = {"XT": XT, "WT": WT,
              "We": np.ascontiguousarray(np.broadcast_to(We, (128, JPC)))}
        if has_bin:
            rows = jp_d * OUT_DIM + c * UPC + jp_u       # global w_in row per j'
            im["binb"] = np.ascontiguousarray(
                np.broadcast_to((b_in[rows] * bcol).astype(np.float32), (128, JPC)))
        if has_bout:
            # bout[w*64 + s*8 + q] = b_out[v], v = q*256 + c*32 + 8w + s
            wq = np.arange(NCHUNK * YW)
            wi, si, qi = wq // YW, (wq % YW) // 8, wq % 8
            vv = qi * (OUT_DIM // DPC) + c * (UPC // DPC) + 8 * wi + si
            im["bout"] = np.ascontiguousarray(np.broadcast_to(b_out[vv], (128, NCHUNK * YW)))
        in_maps.append(im)

    import os
    trace = bool(os.environ.get("KERNEL_TRACE"))
    last_err = None
    for _attempt in range(3):
        try:
            res = run_bass_kernel_spmd(nc, in_maps, list(range(NCORES)), trace=trace)
            break
        except Exception as err:  # rare transient device fault on first execute
            last_err = err
            import time as _time
            _time.sleep(2.0)
    else:
        raise last_err
    LAST_RESULTS = res

    # Y[b, w, s*8+q] (per core) -> y[b, q*256 + c*32 + 8w + s]
    Yc = np.stack([res.results[c]["Y"] for c in range(NCORES)], axis=0)  # [8, B, NCHUNK, 64]
    Yc = Yc.reshape(NCORES, B, NCHUNK, 8, 8)             # [c, b, w, s, q]
    y = Yc.transpose(1, 4, 0, 2, 3).reshape(B, OUT_DIM)  # [b, q, c, w, s] -> v = q*256+c*32+8w+s
    return np.ascontiguousarray(y)
